# revision 15
# baseline (speedup 1.0000x reference)
"""Trainium2 Bass kernel for nn_Colar_static (retrieval_knn).

Strategy: data-parallel over batch B across 8 cores; prototype
projections (Ek/Ev) replicated per core. Everything on device runs in
a "transposed" orientation with the batch on the free dimension and
channels / prototype-columns on partitions, so that:
  - k-row norms / softmax sums are PE ones-matmuls (partition reduce),
  - Ek column norms and per-(k,n) exp scaling are per-partition scalars,
  - no on-device transposes are needed anywhere.
SBUF singles are created in reverse order of death (LIFO pool stack).
"""

import os
import sys

for _p in ("/opt/trn_rl_repo", "/opt/pypackages"):
    if _p not in sys.path:
        sys.path.append(_p)

import numpy as np
import ml_dtypes

import concourse.bass as bass
import concourse.mybir as mybir
import concourse.tile as tile
from concourse import bacc
from concourse import bass_utils

B, T, CH, C, N, K = 4096, 8, 2048, 1024, 512, 5
NCORES = 8
BL = B // NCORES            # 512 batch rows per core
KN = K * N                  # 2560 prototype columns
P = 128
NT_I = CH // P              # 16 contraction tiles (input channels)
NT_C = C // P               # 8 tiles over C
NT_KN = KN // P             # 20 tiles over K*N
NT_KV = 2 * C // P          # 16 tiles over [k|v] output channels
TPK = NT_KN // K            # 4 kn-tiles per prototype
EPS = 1e-8

F32 = mybir.dt.float32
BF16 = mybir.dt.bfloat16
AF = mybir.ActivationFunctionType
MUL = mybir.AluOpType.mult
ADD = mybir.AluOpType.add

_CACHE = {}


def _build_nc():
    PH = int(os.environ.get("KPHASES", "9"))
    KA1 = int(os.environ.get("KA1", "9"))
    nc = bacc.Bacc(None, target_bir_lowering=False, debug=False)

    xT = nc.dram_tensor("xT", [CH, BL], BF16, kind="ExternalInput")
    wkvT = nc.dram_tensor("wkvT", [CH, 2 * C], BF16, kind="ExternalInput")
    wekT = nc.dram_tensor("wekT", [CH, C], BF16, kind="ExternalInput")
    wevT = nc.dram_tensor("wevT", [CH, C], BF16, kind="ExternalInput")
    statf = nc.dram_tensor("statf", [CH, KN], BF16, kind="ExternalInput")
    bek = nc.dram_tensor("bek", [P, NT_C], F32, kind="ExternalInput")
    bkv = nc.dram_tensor("bkv", [P, NT_KV], F32, kind="ExternalInput")
    bev = nc.dram_tensor("bev", [P, C], F32, kind="ExternalInput")
    wwb = nc.dram_tensor("wwb", [P, C], BF16, kind="ExternalInput")
    wout = nc.dram_tensor("wout", [P, NT_KV * K], BF16, kind="ExternalInput")
    bws = nc.dram_tensor("bws", [1, 1], F32, kind="ExternalInput")
    boutt = nc.dram_tensor("boutt", [K, 1], F32, kind="ExternalInput")
    outT = nc.dram_tensor("outT", [K, BL], F32, kind="ExternalOutput")
    # DRAM scratch as ExternalOutputs: Internal DRAM tiles are compiled with
    # physical addresses (--mem-mode=physical) and wedge the device when the
    # NEFF is loaded via the PJRT/axon path; External allocations relocate.
    evspill = nc.dram_tensor("evs", [NT_KN, P, C], BF16, kind="ExternalOutput")
    invbounce = nc.dram_tensor("invb", [1, KN], F32, kind="ExternalOutput")

    tc_cm = tile.TileContext(nc)
    tc = tc_cm.__enter__()
    if True:
        if True:
            # ---- persistents (die at the very end), bottom of pool stack
            ones_col, _f1 = tc.tile([P, 1], BF16, name="ones_col")
            nc.any.memset(ones_col[:], 1.0)
            ones_row, _f2 = tc.tile([1, P], F32, name="ones_row")
            nc.any.memset(ones_row[:], 1.0)
            bek_sb, _f3 = tc.tile([P, NT_C], F32, name="bek_sb")
            nc.sync.dma_start(bek_sb[:], bek[:])
            bkv_sb, _f4 = tc.tile([P, NT_KV], F32, name="bkv_sb")
            nc.sync.dma_start(bkv_sb[:], bkv[:])
            bw_sb, _f5 = tc.tile([1, 1], F32, name="bw_sb")
            nc.sync.dma_start(bw_sb[:], bws[:])
            bout_sb, _f6 = tc.tile([K, 1], F32, name="bout_sb")
            nc.sync.dma_start(bout_sb[:], boutt[:])
            wo_sb, _f7 = tc.tile([P, NT_KV * K], BF16, name="wo_sb")
            nc.sync.dma_start(wo_sb[:], wout[:])

            # dies OUT-end
            vr_all, f_vr = tc.tile([P, NT_C, BL], BF16, name="vr_all")
            fr_all, f_fr = tc.tile([P, NT_C, BL], BF16, name="fr_all")
            # dies GATE-end (written in A2)
            wevA, f_wevA = tc.tile([P, NT_KN], F32, name="wevA")
            wevB, f_wevB = tc.tile([P, NT_KN], F32, name="wevB")
            # dies FE-end (written in fused sim/gate phase)
            wf_all, f_wf = tc.tile([P, NT_KN, BL], BF16, name="wf_all")
            # dies SIM-end (written in A1)
            ek_all, f_ek = tc.tile([P, NT_C, KN], BF16, name="ek_all")
            # dies A1-end (used by A2 as lhsT and A1 as rhs)
            st_all, f_st = tc.tile([P, NT_I, KN], BF16, name="st_all")
            for i in range(NT_I):
                nc.sync.dma_start(st_all[:, i, :], statf[i * P:(i + 1) * P, :])

            # ============ Phase A2: EvT[kn, c] -> DRAM spill, wEv =========
            bev_sb, f_bev = tc.tile([P, C], F32, name="bev_sb")
            nc.sync.dma_start(bev_sb[:], bev[:])
            ww_sb, f_ww = tc.tile([P, C], BF16, name="ww_sb")
            nc.sync.dma_start(ww_sb[:], wwb[:])

            with tc.tile_pool(name="a2w", bufs=3) as a2w, \
                 tc.tile_pool(name="wvp", bufs=1) as wvp, \
                 tc.tile_pool(name="pa2", bufs=3, space="PSUM") as pa2:
                for cc in range(2):
                    wv_h = wvp.tile([P, NT_I, 512], BF16, tag="wvh")
                    for i in range(NT_I):
                        nc.sync.dma_start(
                            wv_h[:, i, :],
                            wevT[i * P:(i + 1) * P, cc * 512:(cc + 1) * 512])
                    wev_half = wevA if cc == 0 else wevB
                    for kt in range(NT_KN):
                        ps = pa2.tile([P, 512], F32, tag="a2ps")
                        for i in range(NT_I):
                            nc.tensor.matmul(
                                ps[:],
                                st_all[:, i, kt * P:(kt + 1) * P],
                                wv_h[:, i, :],
                                start=(i == 0), stop=(i == NT_I - 1))
                        evt_bf = a2w.tile([P, 512], BF16, tag="evtbf")
                        nc.vector.tensor_add(
                            evt_bf[:], ps[:],
                            bev_sb[:, cc * 512:(cc + 1) * 512])
                        scr = a2w.tile([P, 512], BF16, tag="a2scr")
                        nc.vector.tensor_mul(
                            scr[:], evt_bf[:],
                            ww_sb[:, cc * 512:(cc + 1) * 512])
                        nc.vector.tensor_reduce(
                            wev_half[:, kt:kt + 1], scr[:],
                            axis=mybir.AxisListType.X, op=ADD)
                        nc.sync.dma_start(
                            evspill[kt, :, cc * 512:(cc + 1) * 512],
                            evt_bf[:])
            f_ww()
            f_bev()
            if PH < 2:
                f_st(); f_ek(); f_wf(); f_wevB(); f_wevA(); f_fr(); f_vr()
                _f7(); _f6(); _f5(); _f4(); _f3(); _f2(); _f1()
                tc_cm.__exit__(None, None, None)
                nc.compile()
                return nc

            # ============ Phase A1: Ek[c, kn] + invnormE -> DRAM bounce ===
            NCH = KN // 512
            with tc.tile_pool(name="a1w", bufs=3) as a1w, \
                 tc.tile_pool(name="wep", bufs=1) as wep, \
                 tc.tile_pool(name="pa1", bufs=2, space="PSUM") as pa1, \
                 tc.tile_pool(name="pss", bufs=1, space="PSUM") as pss:
                ss = [pss.tile([1, 512], F32, name=f"ss{j}") for j in range(NCH)]
                if KA1 < 1:
                    for j in range(NCH):
                        nc.vector.memset(ss[j], 0.0)
                for wh in range(2):
                    we_h = wep.tile([P, NT_I, 512], BF16, tag="weh")
                    for i in range(NT_I):
                        nc.sync.dma_start(
                            we_h[:, i, :],
                            wekT[i * P:(i + 1) * P, wh * 512:(wh + 1) * 512])
                    for ml in range(NT_C // 2):
                        m = wh * (NT_C // 2) + ml
                        for nch in range(NCH):
                            ps = pa1.tile([P, 512], F32, tag="a1ps")
                            for i in range(NT_I):
                                nc.tensor.matmul(
                                    ps[:],
                                    we_h[:, i, ml * P:(ml + 1) * P],
                                    st_all[:, i, nch * 512:(nch + 1) * 512],
                                    start=(i == 0), stop=(i == NT_I - 1))
                            nc.scalar.activation(
                                ek_all[:, m, nch * 512:(nch + 1) * 512],
                                ps[:], AF.Identity, bias=bek_sb[:, m:m + 1])
                            if KA1 >= 1:
                                sq = a1w.tile([P, 512], BF16, tag="a1sq")
                                nc.scalar.activation(
                                    sq[:], ps[:], AF.Square,
                                    bias=bek_sb[:, m:m + 1])
                                nc.tensor.matmul(
                                    ss[nch], ones_col[:], sq[:],
                                    start=(m == 0), stop=(m == NT_C - 1))
                for j in range(NCH):
                    if KA1 < 1:
                        break
                    if KA1 < 2:
                        tmp = a1w.tile([1, 512], F32, tag="nrow")
                        nc.scalar.copy(tmp[:], ss[j])
                        if int(os.environ.get("KINVDMA", "1")):
                            nc.sync.dma_start(
                                invbounce[0:1, j * 512:(j + 1) * 512],
                                tmp[0:1, :])
                        continue
                    nrow = a1w.tile([1, 512], F32, tag="nrow")
                    nc.scalar.sqrt(nrow[:], ss[j])
                    nrow2 = a1w.tile([1, 512], F32, tag="nrow2")
                    nc.vector.tensor_scalar_max(nrow2[:], nrow[:], EPS)
                    invrow = a1w.tile([1, 512], F32, tag="invrow")
                    nc.vector.reciprocal(invrow[:], nrow2[:])
                    nc.sync.dma_start(invbounce[0:1, j * 512:(j + 1) * 512],
                                      invrow[0:1, :])
            f_st()
            if PH < 3:
                f_ek(); f_wf(); f_wevB(); f_wevA(); f_fr(); f_vr()
                _f7(); _f6(); _f5(); _f4(); _f3(); _f2(); _f1()
                tc_cm.__exit__(None, None, None)
                nc.compile()
                return nc

            # ============ Phase KV: normalized kT, relu(vT) ==============
            # creation order = reverse death: kn_all & inv_col die SIM-end,
            # kT/sqk die after the kn mult, xp dies when kv matmuls finish.
            kn_all, f_kn = tc.tile([P, NT_C, BL], BF16, name="kn_all")
            inv_col, f_inv = tc.tile([P, NT_KN], F32, name="inv_col")
            nc.sync.dma_start(
                inv_col[:], invbounce[0, :].rearrange("(j p) -> p j", p=P))
            kT_all, f_kT = tc.tile([P, NT_C, BL], F32, name="kT_all")
            sqk_all, f_sqk = tc.tile([P, NT_C, BL], BF16, name="sqk_all")
            xp_all, f_xp = tc.tile([P, NT_I, BL], BF16, name="xp_all")
            for i in range(NT_I):
                nc.sync.dma_start(xp_all[:, i, :], xT[i * P:(i + 1) * P, :])

            with tc.tile_pool(name="wkvp", bufs=2) as wkvp, \
                 tc.tile_pool(name="pkv", bufs=2, space="PSUM") as pkv:
                for mg in range(4):
                    kv_ps = [pkv.tile([P, BL], F32, tag=f"kvps{q}",
                                      name=f"kvps{mg}_{q}")
                             for q in range(4)]
                    for i in range(NT_I):
                        wp = wkvp.tile([P, 512], BF16, tag="wp")
                        nc.sync.dma_start(
                            wp[:], wkvT[i * P:(i + 1) * P,
                                        mg * 512:(mg + 1) * 512])
                        for q in range(4):
                            nc.tensor.matmul(
                                kv_ps[q], wp[:, q * P:(q + 1) * P],
                                xp_all[:, i, :],
                                start=(i == 0), stop=(i == NT_I - 1))
                    for q in range(4):
                        m = mg * 4 + q
                        if m < NT_C:
                            nc.scalar.activation(
                                kT_all[:, m, :], kv_ps[q], AF.Identity,
                                bias=bkv_sb[:, m:m + 1])
                            nc.scalar.activation(
                                sqk_all[:, m, :], kv_ps[q], AF.Square,
                                bias=bkv_sb[:, m:m + 1])
                        else:
                            nc.scalar.activation(
                                vr_all[:, m - NT_C, :], kv_ps[q], AF.Relu,
                                bias=bkv_sb[:, m:m + 1])
            f_xp()

            with tc.tile_pool(name="kvw", bufs=2) as kvw, \
                 tc.tile_pool(name="pssk", bufs=1, space="PSUM") as pssk, \
                 tc.tile_pool(name="pbc", bufs=1, space="PSUM") as pbc:
                ssk = pssk.tile([1, BL], F32)
                for m in range(NT_C):
                    nc.tensor.matmul(ssk[:], ones_col[:], sqk_all[:, m, :],
                                     start=(m == 0), stop=(m == NT_C - 1))
                nk = kvw.tile([1, BL], F32, tag="nk")
                nc.scalar.sqrt(nk[:], ssk[:])
                nk2 = kvw.tile([1, BL], F32, tag="nk2")
                nc.vector.tensor_scalar_max(nk2[:], nk[:], EPS)
                invk = kvw.tile([1, BL], F32, tag="invk")
                nc.vector.reciprocal(invk[:], nk2[:])
                bc = pbc.tile([P, BL], F32)
                nc.tensor.matmul(bc[:], ones_row[:], invk[:])
                for m in range(NT_C):
                    nc.vector.tensor_mul(kn_all[:, m, :], kT_all[:, m, :],
                                         bc[:])
            f_sqk()
            f_kT()
            if PH < 4:
                nc.sync.dma_start(evspill[0, :, 0:BL], kn_all[:, 0, :])
                f_xp2_unused = None
                f_inv(); f_kn(); f_ek(); f_wf(); f_wevB(); f_wevA(); f_fr(); f_vr()
                _f7(); _f6(); _f5(); _f4(); _f3(); _f2(); _f1()
                tc_cm.__exit__(None, None, None)
                nc.compile()
                return nc

            # ============ Fused SIM + GATE + WF ==========================
            with tc.tile_pool(name="gw", bufs=2) as gw, \
                 tc.tile_pool(name="esw", bufs=8) as esw, \
                 tc.tile_pool(name="psim", bufs=3, space="PSUM") as psim, \
                 tc.tile_pool(name="pg", bufs=1, space="PSUM") as pg, \
                 tc.tile_pool(name="pbc2", bufs=2, space="PSUM") as pbc2:
                wev_sum = gw.tile([P, NT_KN], F32, tag="wevsum")
                nc.vector.tensor_add(wev_sum[:], wevA[:], wevB[:])
                wev_bf = gw.tile([P, NT_KN], BF16, tag="wevbf")
                nc.vector.tensor_copy(wev_bf[:], wev_sum[:])
                for k in range(K):
                    gse = pg.tile([1, BL], F32, tag="gse")
                    gtg = pg.tile([1, BL], F32, tag="gtg")
                    es_list = []
                    for j in range(TPK):
                        kt = k * TPK + j
                        ps = psim.tile([P, BL], F32, tag="simps")
                        for m in range(NT_C):
                            nc.tensor.matmul(
                                ps[:], ek_all[:, m, kt * P:(kt + 1) * P],
                                kn_all[:, m, :],
                                start=(m == 0), stop=(m == NT_C - 1))
                        es = esw.tile([P, BL], BF16, tag="esw")
                        nc.scalar.activation(es[:], ps[:], AF.Exp,
                                             scale=inv_col[:, kt:kt + 1])
                        es_list.append(es)
                        nc.tensor.matmul(gse[:], ones_col[:], es[:],
                                         start=(j == 0), stop=(j == TPK - 1))
                        nc.tensor.matmul(gtg[:], wev_bf[:, kt:kt + 1], es[:],
                                         start=(j == 0), stop=(j == TPK - 1))
                    rs = gw.tile([1, BL], F32, tag="rs")
                    nc.vector.reciprocal(rs[:], gse[:])
                    tg = gw.tile([1, BL], F32, tag="tg")
                    nc.vector.tensor_mul(tg[:], gtg[:], rs[:])
                    fwk = gw.tile([1, BL], F32, tag="fwk")
                    nc.scalar.activation(fwk[:], tg[:], AF.Sigmoid,
                                         bias=bw_sb[0:1, 0:1])
                    sk = gw.tile([1, BL], F32, tag="sk")
                    nc.vector.tensor_mul(sk[:], fwk[:], rs[:])
                    bcs = pbc2.tile([P, BL], F32, tag="bcs")
                    nc.tensor.matmul(bcs[:], ones_row[:], sk[:])
                    bcs_sb = gw.tile([P, BL], BF16, tag="bcssb")
                    nc.scalar.copy(bcs_sb[:], bcs[:])
                    for j in range(TPK):
                        kt = k * TPK + j
                        nc.vector.tensor_mul(wf_all[:, kt, :], es_list[j],
                                             bcs_sb[:])
            f_inv()
            f_kn()
            f_ek()
            if PH < 5:
                nc.sync.dma_start(evspill[0, :, 0:BL], wf_all[:, 0, :])
                f_wf(); f_wevB(); f_wevA(); f_fr(); f_vr()
                _f7(); _f6(); _f5(); _f4(); _f3(); _f2(); _f1()
                tc_cm.__exit__(None, None, None)
                nc.compile()
                return nc

            # ============ Phase FE ========================================
            evt_all, f_evt = tc.tile([P, NT_KN, C], BF16, name="evt_all")
            for kt in range(NT_KN):
                nc.sync.dma_start(evt_all[:, kt, :], evspill[kt])
            with tc.tile_pool(name="pfe", bufs=3, space="PSUM") as pfe:
                for mc in range(NT_C):
                    ps = pfe.tile([P, BL], F32, tag="feps")
                    for kt in range(NT_KN):
                        nc.tensor.matmul(
                            ps[:], evt_all[:, kt, mc * P:(mc + 1) * P],
                            wf_all[:, kt, :],
                            start=(kt == 0), stop=(kt == NT_KN - 1))
                    nc.scalar.activation(fr_all[:, mc, :], ps[:], AF.Relu)
            f_evt()
            f_wf()
            f_wevB()
            f_wevA()

            # ============ Phase OUT =======================================
            with tc.tile_pool(name="ow", bufs=1) as ow, \
                 tc.tile_pool(name="pout", bufs=1, space="PSUM") as pout:
                po = pout.tile([K, BL], F32)
                for j in range(NT_KV):
                    rhs = vr_all[:, j, :] if j < NT_C else \
                        fr_all[:, j - NT_C, :]
                    nc.tensor.matmul(po[:], wo_sb[:, j * K:(j + 1) * K], rhs,
                                     start=(j == 0), stop=(j == NT_KV - 1))
                osb = ow.tile([K, BL], F32)
                nc.scalar.activation(osb[:], po[:], AF.Identity,
                                     bias=bout_sb[:])
                nc.sync.dma_start(outT[:], osb[:])
            f_fr()
            f_vr()
            _f7()
            _f6()
            _f5()
            _f4()
            _f3()
            _f2()
            _f1()

    tc_cm.__exit__(None, None, None)
    nc.compile()
    return nc


def _host_prep(inputs):
    bf = ml_dtypes.bfloat16
    x_last = np.asarray(inputs["x"])[:, -1, :]  # [B, CH] f32
    shared = {
        "wkvT": np.ascontiguousarray(
            np.concatenate([inputs["Wk"], inputs["Wv"]], axis=0).T
        ).astype(bf),
        "wekT": np.ascontiguousarray(np.asarray(inputs["WEk"]).T).astype(bf),
        "wevT": np.ascontiguousarray(np.asarray(inputs["WEv"]).T).astype(bf),
        "statf": np.ascontiguousarray(
            np.asarray(inputs["static"]).transpose(1, 0, 2).reshape(CH, KN)
        ).astype(bf),
        "bek": np.ascontiguousarray(
            np.asarray(inputs["bEk"]).reshape(NT_C, P).T),
        "bkv": np.ascontiguousarray(
            np.concatenate([inputs["bk"], inputs["bv"]]).reshape(NT_KV, P).T),
        "bev": np.ascontiguousarray(
            np.broadcast_to(np.asarray(inputs["bEv"]), (P, C))),
        "wwb": np.ascontiguousarray(
            np.broadcast_to(np.asarray(inputs["Ww"])[0], (P, C))).astype(bf),
        "wout": np.ascontiguousarray(
            np.asarray(inputs["Wout"]).T.reshape(NT_KV, P, K)
            .transpose(1, 0, 2).reshape(P, NT_KV * K)).astype(bf),
        "bws": np.asarray(inputs["bw"], dtype=np.float32).reshape(1, 1),
        "boutt": np.asarray(inputs["bout"], dtype=np.float32).reshape(K, 1),
    }
    in_maps = []
    for r in range(NCORES):
        m = dict(shared)
        m["xT"] = np.ascontiguousarray(
            x_last[r * BL:(r + 1) * BL].T).astype(bf)
        in_maps.append(m)
    return in_maps


def kernel(**inputs):
    if "nc" not in _CACHE:
        _CACHE["nc"] = _build_nc()
    nc = _CACHE["nc"]
    in_maps = _host_prep(inputs)
    res = bass_utils.run_bass_kernel_spmd(
        nc, in_maps, core_ids=list(range(NCORES)), trace=False)
    out = np.concatenate(
        [res.results[r]["outT"].T for r in range(NCORES)], axis=0)
    return np.ascontiguousarray(out[:, :, None], dtype=np.float32)
